# revision 34
# baseline (speedup 1.0000x reference)
"""Trainium2 Bass kernel for nn_MinMaxLayer (ragged AdaptiveMaxPool1d min+max+sort).

Strategy (pure data parallel over batch, 8 cores):
  Host: for each row, compute the R=5 adaptive windows [a_r, b_r) over the
  first L elements; copy each window (padded up to a K=128 block boundary
  with duplicates of its first element) into a fixed-width slot. Two rows
  are bin-packed per slot so ~half of the padded input volume is skipped
  (lengths are uniform in [5, 8192], so pairs of sorted rows fill a slot of
  ~8960 columns). Slots are dealt 128-per-group to the 8 cores.

  Device (per core, per group of 128 slots):
    1. chunked DMA of the [128, W] slot tile (double-buffered),
    2. one segmented reduce_max and one reduce_min(negate=True) pass
       -> per-block max / negated-min partials [128, W/K],
    3. per (row-in-slot, window): add a host-built additive {0, -inf} mask
       (broadcasting the partials x10 via a stride-0 access pattern) and
       segmented-reduce -> the 10 window min/max values per slot,
    4. rank-based sort of the 10 values (pairwise compares + one-hot
       scatter, all width-200 DVE ops),
    5. DMA the sorted [128, 20] out.

  Host: scatter slot rows back to the original batch order.

All min/max/sort operations are exact selections, so the result is
bit-identical to the fp32 reference.

Raw Bass (no Tile framework): this toolchain's walrus accepts at most one
sync-wait per instruction, so synchronization is explicit two-semaphore
producer/consumer counting (dsem: DMA completions x16; vsem: DVE progress).
"""
import numpy as np

R = 5
NCORES = 8
P = 128
K = 128          # block size (columns per partial)
NCHUNK = 5       # column chunks per group for DMA/compute overlap
NBUF = 8         # max chunk buffers (actual count adapts to the schedule)

FMAX = float(np.finfo(np.float32).max)

_nc_cache = {}


# ----------------------------------------------------------------- planning

def _windows(lengths):
    r = np.arange(R, dtype=np.int64)
    L = lengths[:, None].astype(np.int64)
    a = (r[None, :] * L) // R
    b = ((r[None, :] + 1) * L + R - 1) // R
    return a, b


def _plan(lengths):
    nrows = lengths.shape[0]
    a, b = _windows(lengths)
    wblk = -(-(b - a) // K)               # [B, R] blocks per window
    row_blks = wblk.sum(axis=1)           # [B]
    order = np.argsort(-row_blks, kind="stable")
    maxrow = int(row_blks.max())

    def pack(cap_blocks):
        """Two-pointer pairing over rows sorted desc by size."""
        slots = []
        i, j = 0, nrows - 1
        while i <= j:
            ri = order[i]
            if i == j:
                slots.append((ri,))
                break
            rj = order[j]
            if row_blks[ri] + row_blks[rj] <= cap_blocks:
                slots.append((ri, rj))
                i += 1
                j -= 1
            else:
                slots.append((ri,))
                i += 1
        return slots

    best = None
    for G in (1, 2, 4, 8, 16):
        cap_slots = G * NCORES * P
        if 2 * cap_slots >= nrows:
            # minimal Wb with <= cap_slots slots (monotone in Wb)
            lo, hi = maxrow, 2 * maxrow + 1
            while lo < hi:
                mid = (lo + hi) // 2
                if len(pack(mid)) <= cap_slots:
                    hi = mid
                else:
                    lo = mid + 1
            total = cap_slots * lo
            if best is None or total < best[0]:
                best = (total, G, lo)
    _, G, Wb = best
    slots = pack(Wb)

    slot_of_row = np.zeros(nrows, dtype=np.int64)
    rowslot_of_row = np.zeros(nrows, dtype=np.int64)
    base_blk = np.zeros(nrows, dtype=np.int64)
    for s, rows in enumerate(slots):
        off = 0
        for k, ridx in enumerate(rows):
            slot_of_row[ridx] = s
            rowslot_of_row[ridx] = k
            base_blk[ridx] = off
            off += row_blks[ridx]

    cum = np.cumsum(wblk, axis=1)
    win_start_blk = base_blk[:, None] + cum - wblk     # [B, R]
    win_end_blk = base_blk[:, None] + cum

    return dict(a=a, b=b, wblk=wblk, G=G, Wb=Wb, n_slots=len(slots),
                slot_of_row=slot_of_row, rowslot_of_row=rowslot_of_row,
                win_start_blk=win_start_blk, win_end_blk=win_end_blk)


def _pack_inputs(x, plan):
    """Build per-core in_maps: packed data, additive window masks, constants."""
    G, Wb = plan["G"], plan["Wb"]
    W = Wb * K
    S = G * P                              # slots per core
    nrows = x.shape[0]

    xin = np.zeros((NCORES, S, W), dtype=np.float32)
    adm = np.full((NCORES, S, 10 * Wb), -FMAX, dtype=np.float32)

    slot_of_row = plan["slot_of_row"]
    rowslot = plan["rowslot_of_row"]
    ws, we = plan["win_start_blk"], plan["win_end_blk"]
    a, b, wblk = plan["a"], plan["b"], plan["wblk"]

    core_of_slot = slot_of_row // S
    local_of_slot = slot_of_row % S
    for ridx in range(nrows):
        c = core_of_slot[ridx]
        sl = local_of_slot[ridx]
        rs = rowslot[ridx]
        for rr in range(R):
            st, en = a[ridx, rr], b[ridx, rr]
            c0 = ws[ridx, rr] * K
            seg = x[ridx, st:en]
            n = en - st
            xin[c, sl, c0:c0 + n] = seg
            padlen = wblk[ridx, rr] * K - n
            if padlen:
                xin[c, sl, c0 + n:c0 + n + padlen] = seg[0]
            s10 = rs * R + rr
            adm[c, sl, s10 * Wb + ws[ridx, rr]:s10 * Wb + we[ridx, rr]] = 0.0

    # constants: cols 0..99 tri (j<i), 100..109 iota
    tri = (np.arange(10)[None, :] < np.arange(10)[:, None]).astype(np.float32)
    cst_row = np.concatenate([tri.reshape(100), np.arange(10, dtype=np.float32)])
    cst = np.broadcast_to(cst_row, (P, 110)).copy()

    return [
        {"xin": xin[c], "adm": adm[c], "cst": cst}
        for c in range(NCORES)
    ]


# ------------------------------------------------------------------- kernel

def _build(Wb, G, checked=False):
    import sys
    if "/opt/trn_rl_repo" not in sys.path:
        sys.path.insert(0, "/opt/trn_rl_repo")
    from concourse import bass, mybir

    f32 = mybir.dt.float32
    W = Wb * K
    S = G * P
    # short warm-up then even mid-size chunks: few enough ops that the
    # per-op overhead stays small, small enough that the two DMA queues
    # stay ahead of the DVE (0.267 us/block consume vs 0.193 us/block feed)
    def chunk_sizes(total):
        sizes = []
        for warm in (2, 4, 8, 8):
            if sum(sizes) + warm <= total // 2:
                sizes.append(warm)
        rest = total - sum(sizes)
        if rest > 0:
            ntail = max(1, -(-rest // 10))
            tail = rest // ntail
            extra = rest - tail * ntail
            sizes += [tail + (1 if i < extra else 0) for i in range(ntail)]
        return [s for s in sizes if s > 0]

    chunks = []                           # (group, c0, cw, global_idx)
    gi = 0
    for g in range(G):
        c0 = 0
        for cw in chunk_sizes(Wb):
            chunks.append((g, c0, cw, gi))
            c0 += cw
            gi += 1
    nchunks = gi
    nch_per_g = nchunks // G
    cb = max(cw for (_, _, cw, _) in chunks)   # buffer size (blocks)
    nbuf = min(nch_per_g + 1, 8)

    nc = bass.Bass()
    xin = nc.declare_dram_parameter("xin", [S, W], f32, isOutput=False)
    admd = nc.declare_dram_parameter("adm", [S, 10 * Wb], f32, isOutput=False)
    cstd = nc.declare_dram_parameter("cst", [P, 110], f32, isOutput=False)
    yout = nc.declare_dram_parameter("yout", [S, 20], f32, isOutput=True)

    AP = bass.AP
    Alu = mybir.AluOpType
    Ax = mybir.AxisListType

    import contextlib
    with contextlib.ExitStack() as ctx:
        block = ctx.enter_context(nc.Block())
        # One sem per chunk-buffer slot: sub-completions of distinct DMAs
        # interleave on a shared sem, so per-slot counting is the only sound
        # scheme (at most one in-flight DMA per slot via the vsem recycle
        # wait).
        bsem = [ctx.enter_context(nc.semaphore(f"bsem{i}")) for i in range(nbuf)]
        asem = ctx.enter_context(nc.semaphore("asem"))
        csem = ctx.enter_context(nc.semaphore("csem"))
        osem = ctx.enter_context(nc.semaphore("osem"))
        esem = ctx.enter_context(nc.semaphore("esem"))
        xb = [ctx.enter_context(nc.sbuf_tensor(f"xb{i}", [P, cb * K], f32))
              for i in range(nbuf)]
        pmax = ctx.enter_context(nc.sbuf_tensor("pmax", [P, Wb], f32))
        npmin = ctx.enter_context(nc.sbuf_tensor("npmin", [P, Wb], f32))
        adm = ctx.enter_context(nc.sbuf_tensor("adm_sb", [P, 10 * Wb], f32))
        cst = ctx.enter_context(nc.sbuf_tensor("cst_sb", [P, 110], f32))
        rpm = ctx.enter_context(nc.sbuf_tensor("rpm", [P, 10 * Wb], f32))
        gt = ctx.enter_context(nc.sbuf_tensor("gt", [P, 200], f32))
        eq = ctx.enter_context(nc.sbuf_tensor("eq", [P, 200], f32))
        em = ctx.enter_context(nc.sbuf_tensor("em", [P, 200], f32))
        sm = ctx.enter_context(nc.sbuf_tensor("sm", [P, 200], f32))
        rank = ctx.enter_context(nc.sbuf_tensor("rank", [P, 20], f32))
        oh = ctx.enter_context(nc.sbuf_tensor("oh", [P, 200], f32))
        pm = ctx.enter_context(nc.sbuf_tensor("pm", [P, 200], f32))
        v20 = [ctx.enter_context(nc.sbuf_tensor(f"v20_{g}", [P, 20], f32))
               for g in range(G)]
        out20 = [ctx.enter_context(nc.sbuf_tensor(f"out20_{g}", [P, 20], f32))
                 for g in range(G)]

        # esem counts completed DVE data ops (each op then_incs it by 1).
        # Per group: 2 ops per chunk, then 4 level-2 ops, then 8 rank ops.
        ops_per_group = 2 * nch_per_g + 12

        def ops_after_chunk(j):
            return 2 * (j + 1) + ops_per_group * (j // nch_per_g) \
                - 2 * nch_per_g * (j // nch_per_g)

        def ops_after_group(g):
            return (g + 1) * ops_per_group

        # chunks alternate between the two HWDGE rings (sync + scalar) so two
        # transfers stay in flight and completion latency is hidden
        max_j0 = max(jj for (gg, _, _, jj) in chunks if gg == 0)

        def issue_chunks(eng, parity):
            side_issued = False

            def issue_side():
                # side inputs for level-2/rank, after group 0's chunk share
                eng.dma_start(out=adm[:, :], in_=admd[:, :]).then_inc(asem, 16)
                eng.dma_start(out=cst[:, :], in_=cstd[:, :]).then_inc(csem, 16)

            for (g, c0, cw, j) in chunks:
                if j % 2 != parity:
                    continue
                if parity == 1 and not side_issued and j > max_j0:
                    issue_side()
                    side_issued = True
                if j >= nbuf:
                    eng.wait_ge(esem, ops_after_chunk(j - nbuf))
                eng.dma_start(
                    out=xb[j % nbuf][:, :cw * K],
                    in_=xin[g * P:(g + 1) * P, c0 * K:(c0 + cw) * K],
                ).then_inc(bsem[j % nbuf], 16)
            if parity == 1 and not side_issued:
                issue_side()

        @block.scalar
        def _(scalar):
            issue_chunks(scalar, 1)

        @block.sync
        def _(sync):
            issue_chunks(sync, 0)
            for g in range(G):
                sync.wait_ge(esem, ops_after_group(g))
                sync.dma_start(
                    out=yout[g * P:(g + 1) * P, :], in_=out20[g][:, :]
                ).then_inc(osem, 16)

        @block.vector
        def _(vector):
            ecount = [0]

            def chain(inst):
                # every DVE data op bumps esem; dependent ops wait on the
                # running count (the race detector does not credit same-engine
                # program order)
                inst.then_inc(esem, 1)
                ecount[0] += 1
                return inst

            def dep_wait():
                # real DVE executes in order (Tile relies on this); the
                # explicit chain waits exist only to satisfy the CoreSim
                # race detector in checked builds
                if checked and ecount[0]:
                    vector.wait_ge(esem, ecount[0])

            for (g, c0, cw, j) in chunks:
                vector.wait_ge(bsem[j % nbuf], 16 * (j // nbuf + 1))
                if c0 == 0 and g > 0:
                    # group g's first reduce overwrites pmax/npmin that group
                    # g-1's level-2 ops read
                    dep_wait()
                xv = xb[j % nbuf][:, :cw * K].rearrange("p (j k) -> p j k", k=K)
                chain(vector.tensor_reduce(
                    out=pmax[:, c0:c0 + cw], in_=xv, op=Alu.max, axis=Ax.X
                ))
                chain(vector.tensor_reduce(
                    out=npmin[:, c0:c0 + cw], in_=xv, op=Alu.min, axis=Ax.X,
                    negate=True,
                ))

                if c0 + cw < Wb:
                    continue
                if g == 0:
                    vector.wait_ge(asem, 16)
                    vector.wait_ge(csem, 16)
                # ---- level 2 for group g: windowed reduce over partials
                # pmax broadcast x10 (stride-0) + additive mask, seg-reduce
                pmax_b = AP(pmax, 0, [[Wb, P], [0, 10], [1, Wb]])
                npmin_b = AP(npmin, 0, [[Wb, P], [0, 10], [1, Wb]])
                adm_v = adm[:, :].rearrange("p (s w) -> p s w", w=Wb)
                rpm_v = rpm[:, :].rearrange("p (s w) -> p s w", w=Wb)
                # max path -> v20 cols rs*10 + r
                dep_wait()
                chain(vector.tensor_tensor(out=rpm_v, in0=pmax_b, in1=adm_v,
                                           op=Alu.add))
                dep_wait()
                chain(vector.tensor_reduce(
                    out=AP(v20[g], 0, [[20, P], [10, 2], [1, 5]]),
                    in_=rpm_v, op=Alu.max, axis=Ax.X,
                ))
                # min path -> v20 cols rs*10 + 5 + r, negated output
                dep_wait()
                chain(vector.tensor_tensor(out=rpm_v, in0=npmin_b, in1=adm_v,
                                           op=Alu.add))
                dep_wait()
                chain(vector.tensor_reduce(
                    out=AP(v20[g], 5, [[20, P], [10, 2], [1, 5]]),
                    in_=rpm_v, op=Alu.max, axis=Ax.X, negate=True,
                ))

                # ---- rank-based sort of the 10 values per (slot, rowslot)
                a_view = AP(v20[g], 0, [[20, P], [10, 2], [1, 10], [0, 10]])
                b_view = AP(v20[g], 0, [[20, P], [10, 2], [0, 10], [1, 10]])
                tri_view = AP(cst, 0, [[110, P], [0, 2], [10, 10], [1, 10]])
                iota_view = AP(cst, 100, [[110, P], [0, 2], [0, 10], [1, 10]])

                def v4(t):
                    return t[:, :].rearrange("p (x i j) -> p x i j", i=10, j=10)

                dep_wait()
                chain(vector.tensor_tensor(out=v4(gt), in0=a_view, in1=b_view,
                                           op=Alu.is_gt))
                dep_wait()
                chain(vector.tensor_tensor(out=v4(eq), in0=a_view, in1=b_view,
                                           op=Alu.is_equal))
                dep_wait()
                chain(vector.tensor_tensor(out=v4(em), in0=v4(eq), in1=tri_view,
                                           op=Alu.mult))
                dep_wait()
                chain(vector.tensor_tensor(out=v4(sm), in0=v4(gt), in1=v4(em),
                                           op=Alu.add))
                dep_wait()
                chain(vector.tensor_reduce(
                    out=rank[:, :],
                    in_=sm[:, :].rearrange("p (ri j) -> p ri j", j=10),
                    op=Alu.add, axis=Ax.X,
                ))
                rank_b = AP(rank, 0, [[20, P], [10, 2], [1, 10], [0, 10]])
                dep_wait()
                chain(vector.tensor_tensor(out=v4(oh), in0=rank_b, in1=iota_view,
                                           op=Alu.is_equal))
                dep_wait()
                chain(vector.tensor_tensor(out=v4(pm), in0=v4(oh), in1=a_view,
                                           op=Alu.mult))
                pm_view = AP(pm, 0, [[200, P], [100, 2], [1, 10], [10, 10]])
                dep_wait()
                chain(vector.tensor_reduce(
                    out=out20[g][:, :].rearrange("p (row r) -> p row r", r=10),
                    in_=pm_view, op=Alu.add, axis=Ax.X,
                ))

            # drain: wait for all output DMAs so the program doesn't retire
            # with DMAs in flight
            vector.wait_ge(osem, 16 * G)

    return nc


def _get_nc(Wb, G, checked=False):
    key = (Wb, G, checked)
    if key not in _nc_cache:
        _nc_cache[key] = _build(Wb, G, checked)
    return _nc_cache[key]


def _run(inputs, lengths, trace=False, tmpdir=None):
    import sys
    if "/opt/trn_rl_repo" not in sys.path:
        sys.path.insert(0, "/opt/trn_rl_repo")
    from concourse.bass_utils import run_bass_kernel_spmd

    x = np.asarray(inputs, dtype=np.float32)
    L = np.asarray(lengths).astype(np.int64)
    plan = _plan(L)
    nc = _get_nc(plan["Wb"], plan["G"])
    in_maps = _pack_inputs(x, plan)
    res = run_bass_kernel_spmd(nc, in_maps, list(range(NCORES)), trace=trace, tmpdir=tmpdir)

    S = plan["G"] * P
    allout = np.concatenate(
        [np.asarray(res.results[c]["yout"]) for c in range(NCORES)], axis=0
    ).reshape(NCORES * S, 2, 10)
    y = allout[plan["slot_of_row"], plan["rowslot_of_row"]].astype(np.float32)
    return y, res


def kernel(inputs, lengths):
    y, _ = _run(inputs, lengths, trace=False)
    return y
